# revision 21
# baseline (speedup 1.0000x reference)
"""Trainium2 Bass kernel for a small decoder block (nn_Decoder_75849122448079).

Math (N=4096 seq, W=512 width, P=64 proj, H=8 heads, F=2048 ffn):
  masked_mh = softmax(q_m k_m^T / 8) v_m @ w_o_sum      (w_o_sum = sum of H row-blocks of w_o)
  mh        = softmax(q_c k_c^T / 8) v_c @ w_o_sum      (q_c from masked_mh; k_c/v_c from x)
  h   = LN(mh + x) * g + b
  y   = LeakyReLU(h @ w1 + b1) @ w2 + b2
  out = LN(y + h) * g + b

Sharding: data-parallel over sequence rows — each of the 8 cores owns 512 query
rows end-to-end. K/V projections are computed on each core's own row slice and
exchanged with two packed AllGathers (masked K/V on the critical path, cross
K/V overlapped with the masked attention); everything else is local. The host
only slices x, re-lays-out / dtype-casts weights (pure marshalling), and
concatenates outputs.

Attention: scores are built transposed (S^T[k, q] = K Q^T), exp'd on the ACT
engine straight out of PSUM, and the softmax denominator rides along as a
ones-column appended to V, so no partition-axis reduction is ever needed. The
masked branch normalizes A in [q, d] layout; the cross branch defers its
normalization through the w_o_sum matmul into the residual step.
"""

import os

import numpy as np

import concourse.bass as bass
import concourse.bacc as bacc
import concourse.mybir as mybir
import concourse.tile as tile
from concourse.bass_utils import run_bass_kernel_spmd
from concourse.masks import make_identity

N, W, P, H, F = 4096, 512, 64, 8, 2048
# kt owning PE rows 0:64 / 64:128 of score group g (see K^T packing)
KT_TOP = [kt for sg in range(0, 32 // 4, 2) for kt in range(4 * sg, 4 * sg + 4)]
KT_BOT = [kt for sg in range(1, 32 // 4, 2) for kt in range(4 * sg, 4 * sg + 4)]
NCORES = 8
R = N // NCORES          # 512 rows per core
RT = R // 128            # 4 row tiles per core
WC = W // 128            # 4 contraction chunks over width
ST = N // 128            # 32 sequence (key) tiles
FC = F // 128            # 16 ffn-hidden tiles
EPS = 1e-5
LEAKY = 0.01
SCALE = 0.125            # 1/sqrt(P)
SLOT_K = 64 * R          # K^T slice elements per core
SLOT_V = 128 * RT * (P + 1)  # V'(with ones col) slice elements per core
SLOT = SLOT_K + SLOT_V

f32 = mybir.dt.float32
bf16 = mybir.dt.bfloat16

# Compute dtype mode: "f32" (exact, 4 cyc/row) or "bf16" (fast, ~1e-3 rel err).
MODE = os.environ.get("BASS_DECODER_MODE", "bf16")


def build_nc(mode=MODE):
    cd = bf16 if mode == "bf16" else f32
    nc = bacc.Bacc()

    # Weights arrive host-re-laid-out, partition-major (see make_in_maps).
    spec = [("x_rows", [128, RT, W], f32),
            ("x_t", [W, N], cd),
            ("xr_t", [128, WC, R], cd),
            ("w_qm2", [128, WC, 2, P], cd),    # [w_q_m | w_q_m]
            ("w_qc2", [128, WC, 2, P], cd),    # [w_q_c | w_q_c]
            ("w_k2", [128, WC, 2, P], cd),     # [w_k_m | w_k_c]
            ("w_k2s", [128, WC, 2, P], cd),    # [w_k_c | w_k_m]
            ("w_v2", [128, WC, 2, P], cd),     # [w_v_m | w_v_c]
            ("w_o", [64, H, W], cd),
            ("ffn_w1", [128, FC, WC, 128], cd),
            ("ffn_w2", [128, FC, W], cd),
            ("ln_g", [W], f32), ("ln_b", [W], f32),
            ("ffn_b1", [128, FC], f32), ("ffn_b2", [W], f32)]
    t = {}
    for n, s, d in spec:
        t[n] = nc.declare_dram_parameter(n, s, d, isOutput=False)
    t["out"] = nc.declare_dram_parameter("out", [R, W], f32, isOutput=True)

    with tile.TileContext(nc) as tc:
        _build(tc, mode, cd, t)
    return nc


def _row_bcast(ap, parts=128):
    """AP reading a 1-D DRAM tensor replicated across `parts` partitions."""
    a = ap[:]
    return bass.AP(tensor=a.tensor, offset=a.offset, ap=[[0, parts]] + list(a.ap))


def _build(tc, mode, cd, t):
    nc = tc.nc
    mm = nc.tensor.matmul

    def tp(out, in_, ident):  # PE transpose
        mm(out, in_, ident, is_transpose=True)

    # ------------------------------------------------------------------ pools
    from contextlib import ExitStack
    ctx = ExitStack()
    persist = ctx.enter_context(tc.tile_pool(name="persist", bufs=1))
    stream = ctx.enter_context(tc.tile_pool(name="stream", bufs=2))
    wstream = ctx.enter_context(tc.tile_pool(name="wstream", bufs=3))
    small = ctx.enter_context(tc.tile_pool(name="small", bufs=4))
    pt_pool = ctx.enter_context(tc.tile_pool(name="pt_pool", bufs=3))
    dram = ctx.enter_context(tc.tile_pool(name="dram", bufs=1, space="DRAM"))
    ps_big = ctx.enter_context(tc.tile_pool(name="ps_big", bufs=2, space="PSUM"))
    ps_acc = ctx.enter_context(tc.tile_pool(name="ps_acc", bufs=1, space="PSUM"))

    def big(shape, dtype=f32):
        return ps_big.tile(shape, dtype, tag="big", name="bigtile")

    def acc(shape, dtype=f32):
        return ps_acc.tile(shape, dtype, tag="acc", name="acctile")

    # -------------------- critical-path loads first (SP queue): x, then qkv
    xr_nat = persist.tile([128, RT, W], f32)
    nc.sync.dma_start(out=xr_nat, in_=t["x_rows"][:])
    wqm2 = persist.tile([128, WC, 2, P], cd)
    nc.sync.dma_start(out=wqm2, in_=t["w_qm2"][:])
    wqc2 = persist.tile([128, WC, 2, P], cd)
    nc.sync.dma_start(out=wqc2, in_=t["w_qc2"][:])
    wk2 = persist.tile([128, WC, 2, P], cd)
    nc.sync.dma_start(out=wk2, in_=t["w_k2"][:])
    wk2s = persist.tile([128, WC, 2, P], cd)
    nc.sync.dma_start(out=wk2s, in_=t["w_k2s"][:])
    wv2 = persist.tile([128, WC, 2, P], cd)
    nc.sync.dma_start(out=wv2, in_=t["w_v2"][:])

    # --------------------- constants on the ACT HWDGE queue (off the SP path)
    ident = persist.tile([128, 128], cd)
    make_identity(nc, ident)
    if cd == f32:
        ident_f32 = ident
    else:
        ident_f32 = persist.tile([128, 128], f32)
        make_identity(nc, ident_f32)

    eps_t = persist.tile([128, 1], f32)
    nc.vector.memset(eps_t, EPS)

    g_rep = persist.tile([128, W], f32)
    nc.scalar.dma_start(out=g_rep, in_=_row_bcast(t["ln_g"]))
    b_rep = persist.tile([128, W], f32)
    nc.scalar.dma_start(out=b_rep, in_=_row_bcast(t["ln_b"]))
    b2_rep = persist.tile([128, W], f32)
    nc.scalar.dma_start(out=b2_rep, in_=_row_bcast(t["ffn_b2"]))
    b1_sb = persist.tile([128, FC], f32)
    nc.scalar.dma_start(out=b1_sb, in_=t["ffn_b1"][:])

    # w_o_sum[d, w] = sum_h w_o[h*P + d, w]   -> [64, W]
    wo_stage = stream.tile([64, H, W], cd, tag="wo")
    nc.scalar.dma_start(out=wo_stage, in_=t["w_o"][:])
    wos_f32 = persist.tile([64, W], f32)
    nc.vector.tensor_add(wos_f32, wo_stage[:, 0, :], wo_stage[:, 1, :])
    for hh in range(2, H):
        nc.vector.tensor_add(wos_f32, wos_f32, wo_stage[:, hh, :])
    if cd == f32:
        wosum = wos_f32
    else:
        wosum = persist.tile([64, W], cd)
        nc.vector.tensor_copy(wosum, wos_f32)

    # x_rows^T, host-pretransposed
    xrT = persist.tile([128, WC, R], cd)
    nc.sync.dma_start(out=xrT, in_=t["xr_t"][:])

    # ------------- full x^T, host-pretransposed: plain contiguous DMAs
    xT = persist.tile([128, WC, N], cd)
    x_t_re = t["x_t"].rearrange("(c p) n -> p c n", p=128)
    NSG = 4
    for sg in range(NSG):
        nc.sync.dma_start(out=xT[:, :, sg * (N // NSG):(sg + 1) * (N // NSG)],
                          in_=x_t_re[:, :, sg * (N // NSG):(sg + 1) * (N // NSG)])

    # K^T packed for concurrent row-group score matmuls. One projection pass
    # per seq chunk, alternating [km|kc] / [kc|km] weights by chunk parity, so
    # each K tensor fills its top rows from even chunks and bottom from odd.
    # Group g pairs kt KT_TOP[g] (PE rows 0:64) with KT_BOT[g] (rows 64:128).
    G = ST // 2
    kmT = persist.tile([128, G, 128], cd)
    kcT = persist.tile([128, G, 128], cd)
    for sg in range(N // 512):
        ps_k = big([128, 512])
        wk = wk2 if sg % 2 == 0 else wk2s
        for wc in range(WC):
            mm(ps_k, wk[:, wc, :, :], xT[:, wc, sg * 512:(sg + 1) * 512],
               start=(wc == 0), stop=(wc == WC - 1))
        lo, hi = 4 * (sg // 2), 4 * (sg // 2) + 4
        if sg % 2 == 0:  # top rows = km, bottom = kc
            nc.vector.tensor_copy(kmT[0:64, lo:hi, :], ps_k[0:64, :])
            nc.vector.tensor_copy(kcT[64:128, lo:hi, :], ps_k[64:128, :])
        else:            # top rows = kc, bottom = km
            nc.vector.tensor_copy(kcT[0:64, lo:hi, :], ps_k[0:64, :])
            nc.vector.tensor_copy(kmT[64:128, lo:hi, :], ps_k[64:128, :])

    # V for both attentions (natural layout, trailing ones column)
    vm = persist.tile([128, ST, P + 1], cd)
    vc = persist.tile([128, ST, P + 1], cd)
    nc.vector.memset(vm[:, :, P:P + 1], 1.0)
    nc.vector.memset(vc[:, :, P:P + 1], 1.0)
    for st in range(ST):
        ps_v = big([128, 2, P])
        for wc in range(WC):
            mm(ps_v, xT[:, wc, st * 128:(st + 1) * 128], wv2[:, wc, :, :],
               start=(wc == 0), stop=(wc == WC - 1))
        nc.vector.tensor_copy(vm[:, st, 0:P], ps_v[:, 0, :])
        nc.vector.tensor_copy(vc[:, st, 0:P], ps_v[:, 1, :])

    # Q^T for the masked branch, duplicated into both partition halves
    ps_q = big([128, R])
    for wc in range(WC):
        mm(ps_q, wqm2[:, wc, :, :], xrT[:, wc, :], start=(wc == 0), stop=(wc == WC - 1))
    qmT = persist.tile([128, R], cd)
    nc.vector.tensor_copy(qmT, ps_q)

    # FFN weight preload on the ACT HWDGE queue
    w1_all = persist.tile([128, FC, WC, 128], cd)
    nc.scalar.dma_start(out=w1_all, in_=t["ffn_w1"][:])
    w2_all = persist.tile([128, FC, W], cd)
    nc.scalar.dma_start(out=w2_all, in_=t["ffn_w2"][:])

    # ------------------------------------------------------------- attention
    def attention(kT, v, qT, out_name):
        """A'^T = [v | 1]^T softmax_unnorm(qk^T/8)^T  -> [P+1, R] unnormalized."""
        ps_aT = acc([P + 1, R])

        def scores(g):
            # kt=g on PE rows 0-63, kt=16+g on rows 64-127: run concurrently
            sT = big([128, 2, 512])
            mm(sT[:, 0, :], kT[0:64, g, :], qT[0:64, :])
            mm(sT[:, 1, :], kT[64:128, g, :], qT[64:128, :])
            return sT

        sT_prev = scores(0)
        for g in range(1, G + 1):
            sT_next = scores(g) if g < G else None
            ptl = pt_pool.tile([128, 2, 512], cd, tag="pt")
            nc.scalar.activation(ptl, sT_prev, mybir.ActivationFunctionType.Exp,
                                 scale=SCALE)
            for j in range(2):
                kt = (KT_TOP, KT_BOT)[j][g - 1]
                mm(ps_aT, v[:, kt, :], ptl[:, j, :],
                   start=(g == 1 and j == 0), stop=(g == G and j == 1))
            sT_prev = sT_next
        aT_sb = persist.tile([P + 1, R], f32, tag=out_name, name=out_name)
        nc.vector.tensor_copy(aT_sb, ps_aT)
        return aT_sb

    # ---------------------------------------------------------- masked branch
    amT = attention(kmT, vm, qmT, "amT")   # [65, R] unnormalized

    # normalize in [q, d] layout: A = A'[:, :64] / A'[:, 64]
    ps_a4 = big([128, RT, P + 1])
    for qt in range(RT):
        tp(ps_a4[:, qt, :], amT[:, qt * 128:(qt + 1) * 128],
           ident_f32[0:P + 1, 0:P + 1])
    a_m = small.tile([128, RT, P], cd, tag="a_m")
    recip_m = small.tile([128, RT, 1], f32, tag="recip")
    for qt in range(RT):
        nc.vector.reciprocal(recip_m[:, qt, :], ps_a4[:, qt, P:P + 1])
        nc.vector.tensor_scalar_mul(a_m[:, qt, :], ps_a4[:, qt, 0:P],
                                    recip_m[:, qt, :])
    # back to A^T [64, R]
    ps_at2 = big([P, R], cd)
    for qt in range(RT):
        tp(ps_at2[:, qt * 128:(qt + 1) * 128], a_m[:, qt, :], ident)
    amT_n = persist.tile([P, R], cd)
    nc.vector.tensor_copy(amT_n, ps_at2)

    # masked_mh^T [128, WC, R] = w_o_sum^T @ A
    mhT = persist.tile([128, WC, R], cd)
    for wc in range(WC):
        ps_mh = big([128, R])
        mm(ps_mh, wosum[:, wc * 128:(wc + 1) * 128], amT_n)
        nc.vector.tensor_copy(mhT[:, wc, :], ps_mh)

    # ----------------------------------------------------------- cross branch
    ps_qc = big([128, R])
    for wc in range(WC):
        mm(ps_qc, wqc2[:, wc, :, :], mhT[:, wc, :], start=(wc == 0), stop=(wc == WC - 1))
    qcT = persist.tile([128, R], cd)
    nc.vector.tensor_copy(qcT, ps_qc)

    acT = attention(kcT, vc, qcT, "acT")   # [65, R]; row 64 = denominators

    # denominators -> [q, 1] layout, reciprocal
    ps_s1 = big([128, RT, 1])
    for qt in range(RT):
        tp(ps_s1[:, qt, :], acT[P:P + 1, qt * 128:(qt + 1) * 128],
           ident_f32[P:P + 1, P:P + 1])
    rs_c = small.tile([128, RT, 1], f32, tag="rs_c")
    for qt in range(RT):
        nc.vector.reciprocal(rs_c[:, qt, :], ps_s1[:, qt, :])

    if cd == f32:
        acT_cd = acT
    else:
        acT_cd = persist.tile([P + 1, R], cd)
        nc.vector.tensor_copy(acT_cd, acT)

    # ----------------------------------------------- h = LN(mh_c + x) * g + b
    h_f32 = persist.tile([128, RT, W], f32)

    def layer_norm(dst, src):
        """dst = LN(src) * g + b  for [128, W] f32 tiles (may alias)."""
        stats = small.tile([128, 6], f32, tag="stats")
        nc.vector.bn_stats(stats, src)
        mv = small.tile([128, 2], f32, tag="mv")
        nc.vector.bn_aggr(mv, stats)
        nc.scalar.activation(mv[:, 1:2], mv[:, 1:2],
                             mybir.ActivationFunctionType.Sqrt,
                             bias=eps_t, scale=1.0)
        nc.vector.reciprocal(mv[:, 1:2], mv[:, 1:2])
        nc.vector.tensor_scalar(dst, src,
                                scalar1=mv[:, 0:1], scalar2=mv[:, 1:2],
                                op0=mybir.AluOpType.subtract,
                                op1=mybir.AluOpType.mult)
        nc.vector.tensor_mul(dst, dst, g_rep)
        nc.vector.tensor_add(dst, dst, b_rep)

    for qt in range(RT):
        ps_mhc = big([128, W])
        mm(ps_mhc, acT_cd[0:P, qt * 128:(qt + 1) * 128], wosum)
        sum_sb = stream.tile([128, W], f32, tag="sum")
        nc.vector.tensor_scalar_mul(sum_sb, ps_mhc, rs_c[:, qt, :])
        nc.vector.tensor_add(sum_sb, sum_sb, xr_nat[:, qt, :])
        layer_norm(h_f32[:, qt, :], sum_sb)

    if cd == f32:
        h_cd = h_f32
    else:
        h_cd = persist.tile([128, RT, W], cd)
        nc.vector.tensor_copy(h_cd, h_f32)

    # h^T [128, WC, R]
    hT = persist.tile([128, WC, R], cd)
    for qt in range(RT):
        pst = big([128, WC, 128], cd)
        for wc in range(WC):
            tp(pst[:, wc, :], h_cd[:, qt, wc * 128:(wc + 1) * 128], ident)
        nc.vector.tensor_copy(hT[:, :, qt * 128:(qt + 1) * 128], pst)

    # ------------------------------------------------------------------- FFN
    ps_y2 = acc([128, RT, W])          # one psum bank per row tile
    for fc in range(FC):
        ps_y1 = big([128, R])
        for wc in range(WC):
            mm(ps_y1, w1_all[:, fc, wc, :], hT[:, wc, :],
               start=(wc == 0), stop=(wc == WC - 1))
        # LeakyReLU(y1 + b1): parametric relu on the ACT engine
        lT = pt_pool.tile([128, R], cd, tag="lT")
        nc.scalar.activation(lT, ps_y1, mybir.ActivationFunctionType.Prelu,
                             bias=b1_sb[:, fc:fc + 1], scale=1.0, alpha=LEAKY)
        for qt in range(RT):
            mm(ps_y2[:, qt, :], lT[:, qt * 128:(qt + 1) * 128], w2_all[:, fc, :],
               start=(fc == 0), stop=(fc == FC - 1))

    # ------------------------------------------ out = LN(y2 + b2 + h) * g + b
    out_re = t["out"].rearrange("(q p) w -> q p w", p=128)
    for qt in range(RT):
        sum2 = stream.tile([128, W], f32, tag="sum")
        nc.vector.tensor_add(sum2, ps_y2[:, qt, :], h_f32[:, qt, :])
        nc.vector.tensor_add(sum2, sum2, b2_rep)
        layer_norm(sum2, sum2)
        nc.sync.dma_start(out=out_re[qt], in_=sum2)

    ctx.close()


_NC_CACHE = {}


def get_nc(mode=MODE):
    if mode not in _NC_CACHE:
        nc = build_nc(mode)
        nc.finalize()
        _NC_CACHE[mode] = nc
    return _NC_CACHE[mode]


def make_in_maps(inputs, mode=MODE):
    """Slice x per core and re-lay-out / cast weights (pure marshalling)."""
    import ml_dtypes
    wd = ml_dtypes.bfloat16 if mode == "bf16" else np.float32

    def pm(a):  # [(c p), d] -> [p, c, d]  (partition-major for contiguous DMA)
        c = a.shape[0] // 128
        return np.ascontiguousarray(
            a.reshape(c, 128, *a.shape[1:]).transpose(1, 0, 2), dtype=wd)

    f = {k: np.asarray(v, dtype=np.float32) for k, v in inputs.items()}
    shared = {
        "w_qm2": np.ascontiguousarray(
            np.stack([pm(f["w_q_m"]), pm(f["w_q_m"])], axis=2), dtype=wd),
        "w_qc2": np.ascontiguousarray(
            np.stack([pm(f["w_q_c"]), pm(f["w_q_c"])], axis=2), dtype=wd),
        "w_k2": np.ascontiguousarray(
            np.stack([pm(f["w_k_m"]), pm(f["w_k_c"])], axis=2), dtype=wd),
        "w_k2s": np.ascontiguousarray(
            np.stack([pm(f["w_k_c"]), pm(f["w_k_m"])], axis=2), dtype=wd),
        "w_v2": np.ascontiguousarray(
            np.stack([pm(f["w_v_m"]), pm(f["w_v_c"])], axis=2), dtype=wd),
        # w_o [(h p), w] -> [p=64, h, w]
        "w_o": np.ascontiguousarray(
            f["w_o"].reshape(H, P, W).transpose(1, 0, 2), dtype=wd),
        # ffn_w1 [(c p), (fc j)] -> [p, fc, c, j]
        "ffn_w1": np.ascontiguousarray(
            f["ffn_w1"].reshape(WC, 128, FC, 128).transpose(1, 2, 0, 3), dtype=wd),
        # ffn_w2 [(fc p), w] -> [p, fc, w]
        "ffn_w2": np.ascontiguousarray(
            f["ffn_w2"].reshape(FC, 128, W).transpose(1, 0, 2), dtype=wd),
        # ffn_b1 [(fc p)] -> [p, fc]
        "ffn_b1": np.ascontiguousarray(f["ffn_b1"].reshape(FC, 128).T),
        "ln_g": f["ln_g"], "ln_b": f["ln_b"], "ffn_b2": f["ffn_b2"],
    }
    x = f["x"]
    x_cd = x.astype(wd)
    shared["x_t"] = np.ascontiguousarray(x_cd.T)
    in_maps = []
    for c in range(NCORES):
        m = dict(shared)
        xr = x[c * R:(c + 1) * R]  # [R, W] -> [p, q, w]
        m["x_rows"] = np.ascontiguousarray(
            xr.reshape(RT, 128, W).transpose(1, 0, 2))
        # x_rows^T [p, c, q]: xr_t[p, c, q] = xr[q, c*128+p]
        m["xr_t"] = np.ascontiguousarray(
            x_cd.T[:, c * R:(c + 1) * R].reshape(WC, 128, R).transpose(1, 0, 2))
        in_maps.append(m)
    return in_maps


def kernel(**inputs):
    in_maps = make_in_maps(inputs)
    nc = get_nc()
    res = run_bass_kernel_spmd(nc, in_maps, list(range(NCORES)))
    return np.concatenate([res.results[c]["out"] for c in range(NCORES)], axis=0)


# revision 22
# speedup vs baseline: 1.0765x; 1.0765x over previous
"""Trainium2 Bass kernel for a small decoder block (nn_Decoder_75849122448079).

Math (N=4096 seq, W=512 width, P=64 proj, H=8 heads, F=2048 ffn):
  masked_mh = softmax(q_m k_m^T / 8) v_m @ w_o_sum      (w_o_sum = sum of H row-blocks of w_o)
  mh        = softmax(q_c k_c^T / 8) v_c @ w_o_sum      (q_c from masked_mh; k_c/v_c from x)
  h   = LN(mh + x) * g + b
  y   = LeakyReLU(h @ w1 + b1) @ w2 + b2
  out = LN(y + h) * g + b

Sharding: data-parallel over sequence rows — each of the 8 cores owns 512 query
rows end-to-end. K/V projections are computed on each core's own row slice and
exchanged with two packed AllGathers (masked K/V on the critical path, cross
K/V overlapped with the masked attention); everything else is local. The host
only slices x, re-lays-out / dtype-casts weights (pure marshalling), and
concatenates outputs.

Attention: scores are built transposed (S^T[k, q] = K Q^T), exp'd on the ACT
engine straight out of PSUM, and the softmax denominator rides along as a
ones-column appended to V, so no partition-axis reduction is ever needed. The
masked branch normalizes A in [q, d] layout; the cross branch defers its
normalization through the w_o_sum matmul into the residual step.
"""

import os

import numpy as np

import concourse.bass as bass
import concourse.bacc as bacc
import concourse.mybir as mybir
import concourse.tile as tile
from concourse.bass_utils import run_bass_kernel_spmd
from concourse.masks import make_identity

N, W, P, H, F = 4096, 512, 64, 8, 2048
# kt owning PE rows 0:64 / 64:128 of score group g (see K^T packing)
KT_TOP = [kt for sg in range(0, 32 // 4, 2) for kt in range(4 * sg, 4 * sg + 4)]
KT_BOT = [kt for sg in range(1, 32 // 4, 2) for kt in range(4 * sg, 4 * sg + 4)]
NCORES = 8
R = N // NCORES          # 512 rows per core
RT = R // 128            # 4 row tiles per core
WC = W // 128            # 4 contraction chunks over width
ST = N // 128            # 32 sequence (key) tiles
FC = F // 128            # 16 ffn-hidden tiles
EPS = 1e-5
LEAKY = 0.01
SCALE = 0.125            # 1/sqrt(P)
SLOT_K = 64 * R          # K^T slice elements per core
SLOT_V = 128 * RT * (P + 1)  # V'(with ones col) slice elements per core
SLOT = SLOT_K + SLOT_V

f32 = mybir.dt.float32
bf16 = mybir.dt.bfloat16

# Compute dtype mode: "f32" (exact, 4 cyc/row) or "bf16" (fast, ~1e-3 rel err).
MODE = os.environ.get("BASS_DECODER_MODE", "bf16")


def build_nc(mode=MODE):
    cd = bf16 if mode == "bf16" else f32
    nc = bacc.Bacc()

    # Weights arrive host-re-laid-out, partition-major (see make_in_maps).
    spec = [("x_rows", [128, RT, W], f32),
            ("x_t", [W, N], cd),
            ("xr_t", [128, WC, R], cd),
            ("w_qm2", [128, WC, 2, P], cd),    # [w_q_m | w_q_m]
            ("w_qc2", [128, WC, 2, P], cd),    # [w_q_c | w_q_c]
            ("w_k2", [128, WC, 2, P], cd),     # [w_k_m | w_k_c]
            ("w_k2s", [128, WC, 2, P], cd),    # [w_k_c | w_k_m]
            ("w_v2", [128, WC, 2, P], cd),     # [w_v_m | w_v_c]
            ("w_o", [64, H, W], cd),
            ("ffn_w1", [128, FC, WC, 128], cd),
            ("ffn_w2", [128, FC, W], cd),
            ("ln_g", [W], f32), ("ln_b", [W], f32),
            ("ffn_b1", [128, FC], f32), ("ffn_b2", [W], f32)]
    t = {}
    for n, s, d in spec:
        t[n] = nc.declare_dram_parameter(n, s, d, isOutput=False)
    t["out"] = nc.declare_dram_parameter("out", [R, W], f32, isOutput=True)

    with tile.TileContext(nc) as tc:
        _build(tc, mode, cd, t)
    return nc


def _row_bcast(ap, parts=128):
    """AP reading a 1-D DRAM tensor replicated across `parts` partitions."""
    a = ap[:]
    return bass.AP(tensor=a.tensor, offset=a.offset, ap=[[0, parts]] + list(a.ap))


def _build(tc, mode, cd, t):
    nc = tc.nc
    mm = nc.tensor.matmul

    def tp(out, in_, ident):  # PE transpose
        mm(out, in_, ident, is_transpose=True)

    # ------------------------------------------------------------------ pools
    from contextlib import ExitStack
    ctx = ExitStack()
    persist = ctx.enter_context(tc.tile_pool(name="persist", bufs=1))
    stream = ctx.enter_context(tc.tile_pool(name="stream", bufs=2))
    wstream = ctx.enter_context(tc.tile_pool(name="wstream", bufs=3))
    small = ctx.enter_context(tc.tile_pool(name="small", bufs=4))
    pt_pool = ctx.enter_context(tc.tile_pool(name="pt_pool", bufs=3))
    dram = ctx.enter_context(tc.tile_pool(name="dram", bufs=1, space="DRAM"))
    ps_big = ctx.enter_context(tc.tile_pool(name="ps_big", bufs=2, space="PSUM"))
    ps_acc = ctx.enter_context(tc.tile_pool(name="ps_acc", bufs=1, space="PSUM"))

    def big(shape, dtype=f32):
        return ps_big.tile(shape, dtype, tag="big", name="bigtile")

    def acc(shape, dtype=f32):
        return ps_acc.tile(shape, dtype, tag="acc", name="acctile")

    # -------------------- critical-path loads first (SP queue): x, then qkv
    xr_nat = persist.tile([128, RT, W], f32)
    nc.sync.dma_start(out=xr_nat, in_=t["x_rows"][:])
    wqm2 = persist.tile([128, WC, 2, P], cd)
    nc.sync.dma_start(out=wqm2, in_=t["w_qm2"][:])
    wqc2 = persist.tile([128, WC, 2, P], cd)
    nc.sync.dma_start(out=wqc2, in_=t["w_qc2"][:])
    wk2 = persist.tile([128, WC, 2, P], cd)
    nc.sync.dma_start(out=wk2, in_=t["w_k2"][:])
    wk2s = persist.tile([128, WC, 2, P], cd)
    nc.sync.dma_start(out=wk2s, in_=t["w_k2s"][:])
    wv2 = persist.tile([128, WC, 2, P], cd)
    nc.sync.dma_start(out=wv2, in_=t["w_v2"][:])

    # --------------------- constants on the ACT HWDGE queue (off the SP path)
    ident = persist.tile([128, 128], cd)
    make_identity(nc, ident)
    if cd == f32:
        ident_f32 = ident
    else:
        ident_f32 = persist.tile([128, 128], f32)
        make_identity(nc, ident_f32)

    eps_t = persist.tile([128, 1], f32)
    nc.vector.memset(eps_t, EPS)

    g_rep = persist.tile([128, W], f32)
    nc.scalar.dma_start(out=g_rep, in_=_row_bcast(t["ln_g"]))
    b_rep = persist.tile([128, W], f32)
    nc.scalar.dma_start(out=b_rep, in_=_row_bcast(t["ln_b"]))
    b2_rep = persist.tile([128, W], f32)
    nc.scalar.dma_start(out=b2_rep, in_=_row_bcast(t["ffn_b2"]))
    b1_sb = persist.tile([128, FC], f32)
    nc.scalar.dma_start(out=b1_sb, in_=t["ffn_b1"][:])

    # w_o_sum[d, w] = sum_h w_o[h*P + d, w]   -> [64, W]
    wo_stage = stream.tile([64, H, W], cd, tag="wo")
    nc.scalar.dma_start(out=wo_stage, in_=t["w_o"][:])
    wos_f32 = persist.tile([64, W], f32)
    nc.vector.tensor_add(wos_f32, wo_stage[:, 0, :], wo_stage[:, 1, :])
    for hh in range(2, H):
        nc.vector.tensor_add(wos_f32, wos_f32, wo_stage[:, hh, :])
    if cd == f32:
        wosum = wos_f32
    else:
        wosum = persist.tile([64, W], cd)
        nc.vector.tensor_copy(wosum, wos_f32)

    # x_rows^T, host-pretransposed
    xrT = persist.tile([128, WC, R], cd)
    nc.sync.dma_start(out=xrT, in_=t["xr_t"][:])

    # ------------- full x^T, host-pretransposed: plain contiguous DMAs
    xT = persist.tile([128, WC, N], cd)
    x_t_re = t["x_t"].rearrange("(c p) n -> p c n", p=128)
    NSG = 4
    for sg in range(NSG):
        nc.sync.dma_start(out=xT[:, :, sg * (N // NSG):(sg + 1) * (N // NSG)],
                          in_=x_t_re[:, :, sg * (N // NSG):(sg + 1) * (N // NSG)])

    # K^T packed for concurrent row-group score matmuls. One projection pass
    # per seq chunk, alternating [km|kc] / [kc|km] weights by chunk parity, so
    # each K tensor fills its top rows from even chunks and bottom from odd.
    # Group g pairs kt KT_TOP[g] (PE rows 0:64) with KT_BOT[g] (rows 64:128).
    G = ST // 2
    kmT = persist.tile([128, G, 128], cd)
    kcT = persist.tile([128, G, 128], cd)
    for sg in range(N // 512):
        ps_k = big([128, 512])
        wk = wk2 if sg % 2 == 0 else wk2s
        for wc in range(WC):
            mm(ps_k, wk[:, wc, :, :], xT[:, wc, sg * 512:(sg + 1) * 512],
               start=(wc == 0), stop=(wc == WC - 1))
        lo, hi = 4 * (sg // 2), 4 * (sg // 2) + 4
        if sg % 2 == 0:  # top rows = km, bottom = kc
            nc.vector.tensor_copy(kmT[0:64, lo:hi, :], ps_k[0:64, :])
            nc.vector.tensor_copy(kcT[64:128, lo:hi, :], ps_k[64:128, :])
        else:            # top rows = kc, bottom = km
            nc.vector.tensor_copy(kcT[0:64, lo:hi, :], ps_k[0:64, :])
            nc.vector.tensor_copy(kmT[64:128, lo:hi, :], ps_k[64:128, :])

    # V for both attentions (natural layout, trailing ones column)
    vm = persist.tile([128, ST, P + 1], cd)
    vc = persist.tile([128, ST, P + 1], cd)
    nc.vector.memset(vm[:, :, P:P + 1], 1.0)
    nc.vector.memset(vc[:, :, P:P + 1], 1.0)
    for st in range(ST):
        ps_v = big([128, 2, P])
        for wc in range(WC):
            mm(ps_v, xT[:, wc, st * 128:(st + 1) * 128], wv2[:, wc, :, :],
               start=(wc == 0), stop=(wc == WC - 1))
        nc.vector.tensor_copy(vm[:, st, 0:P], ps_v[:, 0, :])
        nc.vector.tensor_copy(vc[:, st, 0:P], ps_v[:, 1, :])

    # Q^T for the masked branch, duplicated into both partition halves
    ps_q = big([128, R])
    for wc in range(WC):
        mm(ps_q, wqm2[:, wc, :, :], xrT[:, wc, :], start=(wc == 0), stop=(wc == WC - 1))
    qmT = persist.tile([128, R], cd)
    qm_copy = nc.vector.tensor_copy(qmT, ps_q)

    # FFN weight preload on the ACT HWDGE queue, held back until the
    # projections finish so it doesn't steal HBM bandwidth from the startup
    from concourse.bass import _add_dep_helper
    w1_all = persist.tile([128, FC, WC, 128], cd)
    d1 = nc.scalar.dma_start(out=w1_all, in_=t["ffn_w1"][:])
    _add_dep_helper(d1.ins, qm_copy.ins, sync=True, reason="delay ffn w1 preload")
    w2_all = persist.tile([128, FC, W], cd)
    d2 = nc.scalar.dma_start(out=w2_all, in_=t["ffn_w2"][:])
    _add_dep_helper(d2.ins, qm_copy.ins, sync=True, reason="delay ffn w2 preload")

    # ------------------------------------------------------------- attention
    def attention(kT, v, qT, out_name):
        """A'^T = [v | 1]^T softmax_unnorm(qk^T/8)^T  -> [P+1, R] unnormalized."""
        ps_aT = acc([P + 1, R])

        def scores(g):
            # kt=g on PE rows 0-63, kt=16+g on rows 64-127: run concurrently
            sT = big([128, 2, 512])
            mm(sT[:, 0, :], kT[0:64, g, :], qT[0:64, :])
            mm(sT[:, 1, :], kT[64:128, g, :], qT[64:128, :])
            return sT

        sT_prev = scores(0)
        for g in range(1, G + 1):
            sT_next = scores(g) if g < G else None
            ptl = pt_pool.tile([128, 2, 512], cd, tag="pt")
            nc.scalar.activation(ptl, sT_prev, mybir.ActivationFunctionType.Exp,
                                 scale=SCALE)
            for j in range(2):
                kt = (KT_TOP, KT_BOT)[j][g - 1]
                mm(ps_aT, v[:, kt, :], ptl[:, j, :],
                   start=(g == 1 and j == 0), stop=(g == G and j == 1))
            sT_prev = sT_next
        aT_sb = persist.tile([P + 1, R], f32, tag=out_name, name=out_name)
        nc.vector.tensor_copy(aT_sb, ps_aT)
        return aT_sb

    # ---------------------------------------------------------- masked branch
    amT = attention(kmT, vm, qmT, "amT")   # [65, R] unnormalized

    # normalize in [q, d] layout: A = A'[:, :64] / A'[:, 64]
    ps_a4 = big([128, RT, P + 1])
    for qt in range(RT):
        tp(ps_a4[:, qt, :], amT[:, qt * 128:(qt + 1) * 128],
           ident_f32[0:P + 1, 0:P + 1])
    a_m = small.tile([128, RT, P], cd, tag="a_m")
    recip_m = small.tile([128, RT, 1], f32, tag="recip")
    for qt in range(RT):
        nc.vector.reciprocal(recip_m[:, qt, :], ps_a4[:, qt, P:P + 1])
        nc.vector.tensor_scalar_mul(a_m[:, qt, :], ps_a4[:, qt, 0:P],
                                    recip_m[:, qt, :])
    # back to A^T [64, R]
    ps_at2 = big([P, R], cd)
    for qt in range(RT):
        tp(ps_at2[:, qt * 128:(qt + 1) * 128], a_m[:, qt, :], ident)
    amT_n = persist.tile([P, R], cd)
    nc.vector.tensor_copy(amT_n, ps_at2)

    # masked_mh^T [128, WC, R] = w_o_sum^T @ A
    mhT = persist.tile([128, WC, R], cd)
    for wc in range(WC):
        ps_mh = big([128, R])
        mm(ps_mh, wosum[:, wc * 128:(wc + 1) * 128], amT_n)
        nc.vector.tensor_copy(mhT[:, wc, :], ps_mh)

    # ----------------------------------------------------------- cross branch
    ps_qc = big([128, R])
    for wc in range(WC):
        mm(ps_qc, wqc2[:, wc, :, :], mhT[:, wc, :], start=(wc == 0), stop=(wc == WC - 1))
    qcT = persist.tile([128, R], cd)
    nc.vector.tensor_copy(qcT, ps_qc)

    acT = attention(kcT, vc, qcT, "acT")   # [65, R]; row 64 = denominators

    # denominators -> [q, 1] layout, reciprocal
    ps_s1 = big([128, RT, 1])
    for qt in range(RT):
        tp(ps_s1[:, qt, :], acT[P:P + 1, qt * 128:(qt + 1) * 128],
           ident_f32[P:P + 1, P:P + 1])
    rs_c = small.tile([128, RT, 1], f32, tag="rs_c")
    for qt in range(RT):
        nc.vector.reciprocal(rs_c[:, qt, :], ps_s1[:, qt, :])

    if cd == f32:
        acT_cd = acT
    else:
        acT_cd = persist.tile([P + 1, R], cd)
        nc.vector.tensor_copy(acT_cd, acT)

    # ----------------------------------------------- h = LN(mh_c + x) * g + b
    h_f32 = persist.tile([128, RT, W], f32)

    def ln_finish(dst, v_sb, ssum):
        """dst = LN(v_sb) * g + b, with sum(v) already in ssum [128, 1]."""
        scr = stream.tile([128, W], f32, tag="scr")
        ss2 = small.tile([128, 1], f32, tag="ss2")
        nc.scalar.activation(scr, v_sb, mybir.ActivationFunctionType.Square,
                             accum_out=ss2)
        m = small.tile([128, 1], f32, tag="m")
        nc.vector.tensor_scalar_mul(m, ssum, 1.0 / W)
        var = small.tile([128, 1], f32, tag="var")
        nc.vector.tensor_mul(var, m, m)
        nc.vector.scalar_tensor_tensor(out=var, in0=ss2, scalar=1.0 / W,
                                       in1=var, op0=mybir.AluOpType.mult,
                                       op1=mybir.AluOpType.subtract)
        nc.scalar.activation(var, var, mybir.ActivationFunctionType.Sqrt,
                             bias=eps_t, scale=1.0)
        nc.vector.reciprocal(var, var)
        nc.vector.tensor_scalar(dst, v_sb, scalar1=m, scalar2=var,
                                op0=mybir.AluOpType.subtract,
                                op1=mybir.AluOpType.mult)
        nc.vector.tensor_mul(dst, dst, g_rep)
        nc.vector.tensor_add(dst, dst, b_rep)

    for qt in range(RT):
        ps_mhc = big([128, W])
        mm(ps_mhc, acT_cd[0:P, qt * 128:(qt + 1) * 128], wosum)
        sum_sb = stream.tile([128, W], f32, tag="sum")
        ssum = small.tile([128, 1], f32, tag="ssum")
        nc.vector.scalar_tensor_tensor(out=sum_sb, in0=ps_mhc,
                                       scalar=rs_c[:, qt, :],
                                       in1=xr_nat[:, qt, :],
                                       op0=mybir.AluOpType.mult,
                                       op1=mybir.AluOpType.add,
                                       accum_out=ssum)
        ln_finish(h_f32[:, qt, :], sum_sb, ssum)

    if cd == f32:
        h_cd = h_f32
    else:
        h_cd = persist.tile([128, RT, W], cd)
        nc.vector.tensor_copy(h_cd, h_f32)

    # h^T [128, WC, R]
    hT = persist.tile([128, WC, R], cd)
    for qt in range(RT):
        pst = big([128, WC, 128], cd)
        for wc in range(WC):
            tp(pst[:, wc, :], h_cd[:, qt, wc * 128:(wc + 1) * 128], ident)
        nc.vector.tensor_copy(hT[:, :, qt * 128:(qt + 1) * 128], pst)

    # ------------------------------------------------------------------- FFN
    hb2 = persist.tile([128, RT, W], f32)
    for qt in range(RT):
        nc.vector.tensor_add(hb2[:, qt, :], h_f32[:, qt, :], b2_rep)

    ps_y2 = acc([128, RT, W])          # one psum bank per row tile
    for fc in range(FC):
        ps_y1 = big([128, R])
        for wc in range(WC):
            mm(ps_y1, w1_all[:, fc, wc, :], hT[:, wc, :],
               start=(wc == 0), stop=(wc == WC - 1))
        # LeakyReLU(y1 + b1): parametric relu on the ACT engine
        lT = pt_pool.tile([128, R], cd, tag="lT")
        nc.scalar.activation(lT, ps_y1, mybir.ActivationFunctionType.Prelu,
                             bias=b1_sb[:, fc:fc + 1], scale=1.0, alpha=LEAKY)
        for qt in range(RT):
            mm(ps_y2[:, qt, :], lT[:, qt * 128:(qt + 1) * 128], w2_all[:, fc, :],
               start=(fc == 0), stop=(fc == FC - 1))

    # ------------------------------------------ out = LN(y2 + b2 + h) * g + b
    out_re = t["out"].rearrange("(q p) w -> q p w", p=128)
    for qt in range(RT):
        sum2 = stream.tile([128, W], f32, tag="sum")
        ssum = small.tile([128, 1], f32, tag="ssum")
        nc.vector.scalar_tensor_tensor(out=sum2, in0=ps_y2[:, qt, :],
                                       scalar=1.0, in1=hb2[:, qt, :],
                                       op0=mybir.AluOpType.mult,
                                       op1=mybir.AluOpType.add,
                                       accum_out=ssum)
        ln_finish(sum2, sum2, ssum)
        nc.sync.dma_start(out=out_re[qt], in_=sum2)

    ctx.close()


_NC_CACHE = {}


def get_nc(mode=MODE):
    if mode not in _NC_CACHE:
        nc = build_nc(mode)
        nc.finalize()
        _NC_CACHE[mode] = nc
    return _NC_CACHE[mode]


def make_in_maps(inputs, mode=MODE):
    """Slice x per core and re-lay-out / cast weights (pure marshalling)."""
    import ml_dtypes
    wd = ml_dtypes.bfloat16 if mode == "bf16" else np.float32

    def pm(a):  # [(c p), d] -> [p, c, d]  (partition-major for contiguous DMA)
        c = a.shape[0] // 128
        return np.ascontiguousarray(
            a.reshape(c, 128, *a.shape[1:]).transpose(1, 0, 2), dtype=wd)

    f = {k: np.asarray(v, dtype=np.float32) for k, v in inputs.items()}
    shared = {
        "w_qm2": np.ascontiguousarray(
            np.stack([pm(f["w_q_m"]), pm(f["w_q_m"])], axis=2), dtype=wd),
        "w_qc2": np.ascontiguousarray(
            np.stack([pm(f["w_q_c"]), pm(f["w_q_c"])], axis=2), dtype=wd),
        "w_k2": np.ascontiguousarray(
            np.stack([pm(f["w_k_m"]), pm(f["w_k_c"])], axis=2), dtype=wd),
        "w_k2s": np.ascontiguousarray(
            np.stack([pm(f["w_k_c"]), pm(f["w_k_m"])], axis=2), dtype=wd),
        "w_v2": np.ascontiguousarray(
            np.stack([pm(f["w_v_m"]), pm(f["w_v_c"])], axis=2), dtype=wd),
        # w_o [(h p), w] -> [p=64, h, w]
        "w_o": np.ascontiguousarray(
            f["w_o"].reshape(H, P, W).transpose(1, 0, 2), dtype=wd),
        # ffn_w1 [(c p), (fc j)] -> [p, fc, c, j]
        "ffn_w1": np.ascontiguousarray(
            f["ffn_w1"].reshape(WC, 128, FC, 128).transpose(1, 2, 0, 3), dtype=wd),
        # ffn_w2 [(fc p), w] -> [p, fc, w]
        "ffn_w2": np.ascontiguousarray(
            f["ffn_w2"].reshape(FC, 128, W).transpose(1, 0, 2), dtype=wd),
        # ffn_b1 [(fc p)] -> [p, fc]
        "ffn_b1": np.ascontiguousarray(f["ffn_b1"].reshape(FC, 128).T),
        "ln_g": f["ln_g"], "ln_b": f["ln_b"], "ffn_b2": f["ffn_b2"],
    }
    x = f["x"]
    x_cd = x.astype(wd)
    shared["x_t"] = np.ascontiguousarray(x_cd.T)
    in_maps = []
    for c in range(NCORES):
        m = dict(shared)
        xr = x[c * R:(c + 1) * R]  # [R, W] -> [p, q, w]
        m["x_rows"] = np.ascontiguousarray(
            xr.reshape(RT, 128, W).transpose(1, 0, 2))
        # x_rows^T [p, c, q]: xr_t[p, c, q] = xr[q, c*128+p]
        m["xr_t"] = np.ascontiguousarray(
            x_cd.T[:, c * R:(c + 1) * R].reshape(WC, 128, R).transpose(1, 0, 2))
        in_maps.append(m)
    return in_maps


def kernel(**inputs):
    in_maps = make_in_maps(inputs)
    nc = get_nc()
    res = run_bass_kernel_spmd(nc, in_maps, list(range(NCORES)))
    return np.concatenate([res.results[c]["out"] for c in range(NCORES)], axis=0)
